# revision 1
# baseline (speedup 1.0000x reference)
"""BitLinear (1-bit packed weights) on 8 TRN2 NeuronCores.

out = x @ W.T, x [64, 4096] f32, W [11008, 4096] in {-1,+1} unpacked from
bp (one byte per int32, MSB-first bits).

Strategy (tensor-parallel, no collectives):
 - shard out_features 11008 -> 8 x 1376 rows of W; x replicated.
 - host: repack bp bytes into dense 16-bit words (pure bit layout change),
   transposed to [word-idx, n] and laid out as one [128, 2752] tile per
   core (both 128-word chunks side by side); permute x to match.
 - device per core (raw Block, manual semaphores):
     DVE: w1 = words & (1<<s)            (one op extracts BOTH chunks' plane)
     DVE/ACT: u = w1 * 2^(1-s) - 1       (arith + cast -> exact {-1,+1} bf16)
     PE: column-tiled pairs - chunk (c=0,o) on array cols 0-63 -> psum
         partitions 0-63, chunk (c=1,o) on cols 64-127 -> partitions 64-127,
         running concurrently; accumulate over o=0..15.
     DVE: merge psum[0:64] + psum[64:128] -> out tile; DMA out.
 - PE warmup: dummy matmuls during the input-DMA wait so HAM un-throttles
   before the real accumulation starts.
"""

import sys

sys.path.insert(0, "/opt/trn_rl_repo")

import ml_dtypes
import numpy as np

import concourse.bass as bass
import concourse.mybir as mybir
from concourse.bass_utils import run_bass_kernel_spmd

OUT_F = 11008
IN_F = 4096
M = 64
NCORES = 8
NSH = OUT_F // NCORES  # 1376 rows of W per core
NSH2 = 2 * NSH  # pair-tile width (both chunks)

PACK = 16  # bits per packed word on device
NW = IN_F // PACK  # packed words along k per W row (256)
NCH = NW // 128  # 128-partition word chunks (2)
NPAIR = PACK  # 16 plane-pairs (o = bit offset in word)
NA = NSH // 2  # 688: output columns per column-tile half
QSPLITS = (512, NA - 512)  # psum n-chunks per half (bank = 512 f32)

_dt_word = {16: mybir.dt.uint16, 32: mybir.dt.uint32}[PACK]
_np_word = {16: "<u2", 32: "<u4"}[PACK]

B1 = 5  # w1 pair buffer depth
B2 = 8  # u pair buffer depth
N_WARMUP = 34  # dummy PE matmuls (N=512) to trip the HAM un-throttle
ACT_CAST = frozenset({1, 3, 5, 7, 9, 11})  # pairs whose cast runs on ACT


def _shift(o):
    # word bit position holding k-offset o (little-endian byte packing,
    # MSB-first bit order inside each byte)
    return 8 * (o // 8) + 7 - (o % 8)


def _build():
    nc = bass.Bass()
    bpt = nc.declare_dram_parameter("bpt", [128, NSH2], _dt_word, isOutput=False)
    xr = nc.declare_dram_parameter(
        "xr", [128, (IN_F // 128) * M], mybir.dt.bfloat16, isOutput=False
    )
    out = nc.declare_dram_parameter("out", [M, NSH], mybir.dt.float32, isOutput=True)

    A = mybir.AluOpType

    # engine program-order bookkeeping
    dve_idx = {}  # ('and'|'cast', o) -> 1-based completion count on DVE
    act_idx = {}  # o -> 1-based completion count on ACT
    di = 0
    ai = 0
    for o in range(NPAIR):
        if _shift(o) == 15:
            di += 1
            dve_idx[("cast", o)] = di
            continue
        di += 1
        dve_idx[("and", o)] = di
        if o in ACT_CAST:
            ai += 1
            act_idx[o] = ai
        else:
            di += 1
            if o == NPAIR - 1:
                dve_idx[("cast_h0", o)] = di
                di += 1
            dve_idx[("cast", o)] = di

    with (
        nc.sbuf_tensor("xb", [128, (IN_F // 128) * M], mybir.dt.bfloat16) as xb,
        nc.sbuf_tensor("btw", [128, NSH2], _dt_word) as btw,
        nc.sbuf_tensor("w1", [128, B1, NSH2], _dt_word) as w1,
        nc.sbuf_tensor("u", [128, B2, NSH2], mybir.dt.bfloat16) as u,
        nc.sbuf_tensor("ot2", [128, NA], mybir.dt.float32) as ot2,
        nc.sbuf_tensor("junk", [128, 512], mybir.dt.bfloat16) as junk,
        nc.sbuf_tensor("scr", [1, 1], mybir.dt.float32) as scr,
        nc.psum_tensor("q0", [128, QSPLITS[0]], mybir.dt.float32) as q0,
        nc.psum_tensor("q1", [128, QSPLITS[1]], mybir.dt.float32) as q1,
        nc.psum_tensor("psw", [M, 512], mybir.dt.float32) as psw,
        nc.semaphore("sq") as sq,
        nc.semaphore("sb") as sb,
        nc.semaphore("sv") as sv,
        nc.semaphore("sa") as sa,
        nc.semaphore("sp") as sp,
        nc.semaphore("scp") as scp,
        nc.semaphore("so") as so,
        nc.semaphore("sdone") as sdone,
        nc.Block() as block,
    ):

        @block.gpsimd
        def _(gpsimd: bass.BassEngine):
            gpsimd.wait_ge(sb, 64)
            gpsimd.dma_start(out=xb[0:64, :], in_=xr[0:64, :]).then_inc(sq, 16)

        @block.sync
        def _(sync: bass.BassEngine):
            sync.dma_start(out=btw[32:64, :], in_=bpt[32:64, :]).then_inc(sb, 16)
            sync.dma_start(out=btw[0:32, :], in_=bpt[0:32, :]).then_inc(sb, 16)
            sync.wait_ge(sb, 64)
            sync.dma_start(out=xb[64:128, :], in_=xr[64:128, :]).then_inc(sq, 16)
            # output: two DMAs, one per column-tile half
            sync.wait_ge(sa, len(ACT_CAST) + 2)
            sync.dma_start(out=out[:, 0:NA], in_=ot2[0:M, :]).then_inc(so, 16)
            sync.wait_ge(scp, 2)
            sync.dma_start(out=out[:, NA:NSH], in_=ot2[M : 2 * M, :]).then_inc(so, 16)
            sync.wait_ge(so, 32)
            sync.wait_ge(sdone, 3)
            # (bass clears kernel sems in its own preamble on each execution)

        @block.vector
        def _(vector: bass.BassEngine):
            vector.wait_ge(sb, 64)
            for o in range(NPAIR):
                s = _shift(o)
                if s == 15:
                    # sign bit via compare: u = (v >= 2^15) - 0.5 in {-.5,+.5};
                    # the matching x blocks are pre-scaled by 2 on the host
                    if o >= B2:
                        vector.wait_ge(sp, o - B2 + 1)
                    vector.tensor_scalar(
                        u[:, o % B2, :],
                        btw[:, :],
                        32768.0,
                        0.5,
                        op0=A.is_ge,
                        op1=A.subtract,
                    ).then_inc(sv)
                    continue
                # w1 slot free? its reader is cast(o-B1)
                if o >= B1 and (o - B1) in ACT_CAST:
                    vector.wait_ge(sa, act_idx[o - B1])
                vector.tensor_scalar(
                    w1[:, o % B1, :], btw[:, :], 1 << s, None, op0=A.bitwise_and
                ).then_inc(sv)
                if o not in ACT_CAST:
                    if o >= B2:
                        vector.wait_ge(sp, o - B2 + 1)
                    if o == NPAIR - 1:
                        # halved cast: c=0 half lands one op earlier so the
                        # PE can start the final pair sooner
                        for h in range(2):
                            vector.tensor_scalar(
                                u[:, o % B2, h * NSH : (h + 1) * NSH],
                                w1[:, o % B1, h * NSH : (h + 1) * NSH],
                                float(2.0 ** (1 - s)),
                                -1.0,
                                op0=A.mult,
                                op1=A.add,
                            ).then_inc(sv)
                    else:
                        vector.tensor_scalar(
                            u[:, o % B2, :],
                            w1[:, o % B1, :],
                            float(2.0 ** (1 - s)),
                            -1.0,
                            op0=A.mult,
                            op1=A.add,
                        ).then_inc(sv)
            # copy the B column-tile halves PSUM -> SBUF
            qs = [q0, q1]
            off = 0
            for j, w in enumerate(QSPLITS):
                vector.wait_ge(sp, NPAIR + j)
                vector.tensor_copy(ot2[M : 2 * M, off : off + w], qs[j][M : 2 * M, :]).then_inc(scp)
                off += w
            vector.nop().then_inc(sdone)

        @block.scalar
        def _(scalar: bass.BassEngine):
            scalar.dma_start(out=btw[96:128, :], in_=bpt[96:128, :]).then_inc(sb, 16)
            scalar.dma_start(out=btw[64:96, :], in_=bpt[64:96, :]).then_inc(sb, 16)
            # touch the ACT path early so the activation table loads during
            # the DMA wait instead of on the first real cast
            scalar.activation(
                scr[:, :], scr[:, :], mybir.ActivationFunctionType.Copy, 0.0, 0.0
            )
            for o in sorted(ACT_CAST):
                s = _shift(o)
                scalar.wait_ge(sv, dve_idx[("and", o)])
                if o >= B2:
                    scalar.wait_ge(sp, o - B2 + 1)
                scalar.activation(
                    u[:, o % B2, :],
                    w1[:, o % B1, :],
                    mybir.ActivationFunctionType.Copy,
                    bias=-1.0,
                    scale=float(2.0 ** (1 - s)),
                ).then_inc(sa)
            # copy the A column-tile halves PSUM -> SBUF
            qs = [q0, q1]
            off = 0
            for j, w in enumerate(QSPLITS):
                scalar.wait_ge(sp, NPAIR + j)
                scalar.activation(
                    ot2[0:M, off : off + w],
                    qs[j][0:M, :],
                    mybir.ActivationFunctionType.Copy,
                    bias=0.0,
                    scale=1.0,
                ).then_inc(sa)
                off += w
            scalar.nop().then_inc(sdone)

        @block.tensor
        def _(tensor: bass.BassEngine):
            # HAM warmup on junk data (no DMA dependency)
            for _i in range(N_WARMUP):
                tensor.matmul(
                    psw[:, :], junk[:, 0:M], junk[:, :], start=True, stop=True
                )
            tensor.wait_ge(sq, 32)
            for o in range(NPAIR):
                if o in ACT_CAST:
                    tensor.wait_ge(sa, act_idx[o])
                elif o == NPAIR - 1:
                    tensor.wait_ge(sv, dve_idx[("cast_h0", o)])
                else:
                    tensor.wait_ge(sv, dve_idx[("cast", o)])
                qs = [q0, q1]
                last_pair = o == NPAIR - 1
                ins = None
                for c in range(NCH):
                    if last_pair and c == 1:
                        tensor.wait_ge(sv, dve_idx[("cast", o)])
                    lh = xb[:, (c * PACK + o) * M : (c * PACK + o + 1) * M]
                    st = o == 0 and c == 0
                    sp_ = last_pair and c == NCH - 1
                    base = c * NSH
                    # explicit weight loads for both column tiles, then
                    # non-self-loading matmuls so A and B stream concurrently
                    tensor.ldweights(lh, tile_position=(0, 0))
                    tensor.ldweights(lh, tile_position=(0, 64))
                    # tile A: output cols [0:NA] on psum partitions 0:64
                    # tile B: output cols [NA:NSH] on partitions 64:128
                    for j, w in enumerate(QSPLITS):
                        off = 512 * j
                        i1 = tensor.matmul(
                            qs[j][0:M, :],
                            lh,
                            u[:, o % B2, base + off : base + off + w],
                            start=st,
                            stop=sp_,
                            tile_position=(0, 0),
                        )
                        i1.ins.ldweights = False
                        ins = tensor.matmul(
                            qs[j][M : 2 * M, :],
                            lh,
                            u[:, o % B2, base + NA + off : base + NA + off + w],
                            start=st,
                            stop=sp_,
                            tile_position=(0, 64),
                        )
                        ins.ins.ldweights = False
                        if sp_:
                            ins.then_inc(sp)  # per-region completion
                if not last_pair:
                    ins.then_inc(sp)
                if o < NPAIR - 3:
                    for _k in range(2):
                        tensor.matmul(
                            psw[:, :], junk[:, 0:M], junk[:, :], start=True, stop=True
                        )
            tensor.nop().then_inc(sdone)

    return nc


def _prep(x, bp):
    x = np.asarray(x, dtype=np.float32)
    bp = np.asarray(bp)
    bytes_ = bp.astype(np.uint8)  # values are 0..255 by construction
    B = bytes_.reshape(OUT_F, IN_F // 8)
    # x[m, k] with k = PACK*(128*c + p) + o  ->  xh[p, (c, o, m)]
    xr4 = np.ascontiguousarray(x.reshape(M, NCH, 128, PACK).transpose(2, 1, 3, 0))
    for o in range(PACK):
        if _shift(o) == 15:
            xr4[:, :, o, :] *= 2.0
    xh = xr4.reshape(128, -1).astype(ml_dtypes.bfloat16)
    in_maps = []
    for cid in range(NCORES):
        Bc = np.ascontiguousarray(B[cid * NSH : (cid + 1) * NSH])  # [1376, 512] u8
        Wd = Bc.view(_np_word)  # [1376, NW] little-endian words
        bptT = np.ascontiguousarray(Wd.T)  # [NW=256, 1376]
        # both 128-word chunks side by side: [128, 2752]
        pair = np.concatenate([bptT[0:128, :], bptT[128:256, :]], axis=1)
        in_maps.append({"bpt": np.ascontiguousarray(pair), "xr": xh})
    return in_maps


def _run(x, bp, trace=False):
    in_maps = _prep(x, bp)
    nc = _build()
    res = run_bass_kernel_spmd(nc, in_maps, list(range(NCORES)), trace=trace)
    outs = [np.asarray(res.results[c]["out"]) for c in range(NCORES)]
    full = np.concatenate(outs, axis=1).astype(np.float32)
    return full, res


def kernel(x, bp):
    out, _ = _run(x, bp, trace=False)
    return out

